# revision 19
# baseline (speedup 1.0000x reference)
"""Trainium2 Bass kernel for AttentionPatcher (GQA attention block, S=2048).

Sharding: 8-way tensor parallel over KV head groups. Core c owns KV head c
and query heads 4c..4c+3: it computes its Q/K/V projections, RoPE, causal
attention, and a full partial o_proj (wo column shard); a per-s-tile
ReduceScatter(add) over the 8 cores then leaves core c with rows
[512c, 512c+512) of the final output, which the host concatenates.

v3: all matmul operands bf16 (adds ~7e-3 rel err vs the 2e-2 gate), wq/wo
SBUF-resident, phases interleaved (QKV / attention / o_proj) with RoPE and
V-transpose work injected into the next phase's matmul stream, PSUM managed
as 2-bank pair tiles (scores pairs share one exp), chunked DMA to keep the
DGE descriptor generators off the critical path, and approximate reciprocal
for the softmax normalizer.
"""
import os
import sys

import numpy as np

if os.path.isdir("/opt/trn_rl_repo") and "/opt/trn_rl_repo" not in sys.path:
    sys.path.insert(0, "/opt/trn_rl_repo")

import ml_dtypes

import concourse.bacc as bacc
import concourse.mybir as mybir
import concourse.tile as tile
from concourse.bass_utils import run_bass_kernel_spmd
from concourse.masks import make_identity

F32 = mybir.dt.float32
F32R = mybir.dt.float32r
BF16 = mybir.dt.bfloat16
ActF = mybir.ActivationFunctionType
Alu = mybir.AluOpType
NPBF = ml_dtypes.bfloat16

H, KV, D, S = 32, 8, 128, 2048
HID = H * D
NCORES = 8
G = H // KV          # query heads per core
ST = 512             # s-tile size
NST = S // ST        # 4 s-tiles
KO = HID // 128      # 32 contraction subtiles
MO = HID // 128      # 32 output row tiles
XC = 4               # x-tile DMA chunk (ko per descriptor)
INV_SQRT_D = 1.0 / float(np.sqrt(D))


def build_nc(with_collective=True):
    nc = bacc.Bacc("TRN2", target_bir_lowering=False, debug=False)

    x = nc.dram_tensor("x", [NST, 128, KO, ST], BF16, kind="ExternalInput")
    wq = nc.dram_tensor("wq", [128, KO, G * 128], BF16, kind="ExternalInput")
    wk = nc.dram_tensor("wk", [128, KO, 128], BF16, kind="ExternalInput")
    wv = nc.dram_tensor("wv", [128, KO, 128], BF16, kind="ExternalInput")
    wo = nc.dram_tensor("wo", [128, MO, G, 128], BF16, kind="ExternalInput")
    bq = nc.dram_tensor("bq", [128, G], F32, kind="ExternalInput")
    bk = nc.dram_tensor("bk", [128, 1], F32, kind="ExternalInput")
    bv = nc.dram_tensor("bv", [128, 1], F32, kind="ExternalInput")
    cos = nc.dram_tensor("cos", [128, S], F32, kind="ExternalInput")
    sin = nc.dram_tensor("sin", [128, S], F32, kind="ExternalInput")
    rot = nc.dram_tensor("rot", [128, 128], F32R, kind="ExternalInput")
    yout = nc.dram_tensor("y", [NCORES // 2, 128, S], BF16,
                          kind="ExternalOutput")

    with tile.TileContext(nc) as tc:
        with (
            tc.tile_pool(name="const", bufs=1) as const,
            tc.tile_pool(name="sb", bufs=3) as sb,
            tc.tile_pool(name="ps", bufs=1, space="PSUM") as ps,
            tc.tile_pool(name="dram", bufs=1, space="DRAM") as dram,
        ):
            # ---- resident weights/tables ----
            wq_sb = const.tile([128, KO, G * 128], BF16)   # 32K/part
            wk_sb = const.tile([128, KO, 128], BF16)       # 8K
            wv_sb = const.tile([128, KO, 128], BF16)       # 8K
            wo_sb = const.tile([128, MO, G, 128], BF16)    # 32K
            # wk/wv/wq chunks are loaded just-in-time inside emit_qkv(0);
            # cos/sin/rot/bias loads are emitted AFTER emit_qkv(0): QKV(0)
            # already needs wq(8M)+x(4M)+wk/wv(2M) and sits right at the HBM
            # bandwidth budget; these are first used at the qkv(0) boundary
            cos_sb = const.tile([128, S], F32)
            sin_sb = const.tile([128, S], F32)
            rot_sb = const.tile([128, 128], F32R)
            bq_sb = const.tile([128, G], F32)
            bk_sb = const.tile([128, 1], F32)
            bv_sb = const.tile([128, 1], F32)

            def emit_tables_load():
                # small tensors first: the qkv(0) boundary evictions read the
                # biases, and the first rope matmul reads rot
                nc.scalar.dma_start(rot_sb[:], rot[:, :])
                nc.scalar.dma_start(bq_sb[:], bq[:, :])
                nc.scalar.dma_start(bk_sb[:], bk[:, :])
                nc.scalar.dma_start(bv_sb[:], bv[:, :])
                nc.scalar.dma_start(cos_sb[:], cos[:, :])
                nc.scalar.dma_start(sin_sb[:], sin[:, :])

            ones_bf = const.tile([128, 128], BF16)
            nc.vector.memset(ones_bf[:], 1.0)
            ident_f = const.tile([128, 128], F32)
            make_identity(nc, ident_f)
            ident_bf = const.tile([128, 128], BF16)
            nc.vector.tensor_copy(ident_bf[:], ident_f[:])
            # static causal masks for the 4 diagonal l-blocks of an s-tile:
            # mask_j[l, s] = 1 where s - l >= 128j else 0
            cmask = const.tile([128, 4, ST], BF16)
            nc.vector.memset(cmask[:], 1.0)
            for j in range(4):
                nc.gpsimd.affine_select(
                    out=cmask[:, j, :], in_=cmask[:, j, :],
                    compare_op=Alu.is_ge, fill=0.0,
                    base=-128 * j, channel_multiplier=-1,
                    pattern=[[1, ST]],
                )

            # ---- resident activations (all bf16) ----
            k_rot = const.tile([128, S], BF16)             # K, (d, l)
            q_rot = const.tile([128, G, S], BF16)          # Q, (d, g, s)
            v_t = const.tile([128, S // 128, 128], BF16)   # V^T
            out_t = const.tile([128, G, S], BF16)          # attn out

            cc_in = dram.tile([NST, MO, 128, ST], BF16)
            cc_out = dram.tile([NST, NCORES // 2, 128, ST], BF16)

            # PSUM discipline: every psum tile is a [128, 2, ST] f32 "pair"
            # (2 banks); the 4 pair bufs cover all 8 banks. Halves act as
            # independent accumulators.
            def pair(name):
                return ps.tile([128, 2, ST], F32, tag="pair", bufs=4,
                               name=name)

            # deferred PE work (rope matmuls / V transposes) injected into
            # the next phase's matmul stream so the PE queue never stalls on
            # the vector-engine eviction chain at a phase boundary
            pending = []

            def drain(n=1):
                for _ in range(n):
                    if pending:
                        pending.pop(0)()

            def rope_cb(raw, dst_ap, sl):
                def cb():
                    ps_r = pair("ps_rot")
                    nc.tensor.matmul(ps_r[:, 0, :], rot_sb[:], raw[:],
                                     start=True, stop=True)
                    t1 = sb.tile([128, ST], F32, tag="rt1", bufs=2)
                    t2 = sb.tile([128, ST], F32, tag="rt2", bufs=2)
                    # split across engines: gpsimd takes the SBUF-only ops
                    nc.gpsimd.tensor_tensor(t1[:], raw[:], cos_sb[:, sl],
                                            Alu.mult)
                    nc.vector.tensor_tensor(t2[:], ps_r[:, 0, :],
                                            sin_sb[:, sl], Alu.mult)
                    nc.gpsimd.tensor_tensor(dst_ap, t1[:], t2[:], Alu.add)
                return cb

            def vt_cb(v_sb, si):
                def cb():
                    ps_t = ps.tile([128, 4, 128], BF16, tag="pair", bufs=4,
                                   name="ps_t")
                    for j in range(4):
                        nc.tensor.transpose(ps_t[:, j, :],
                                            v_sb[:, j * 128:(j + 1) * 128],
                                            ident_bf[:])
                    nc.vector.tensor_copy(v_t[:, si * 4:(si + 1) * 4, :],
                                          ps_t[:])
                return cb

            def emit_qkv(si):
                sl = slice(si * ST, (si + 1) * ST)
                ps_q01 = pair("ps_q01")
                ps_q23 = pair("ps_q23")
                ps_kv = pair("ps_kv")
                ps_q = [ps_q01[:, 0, :], ps_q01[:, 1, :],
                        ps_q23[:, 0, :], ps_q23[:, 1, :]]
                for kc in range(KO // XC):
                    if si == 0:
                        wsl = slice(kc * XC, (kc + 1) * XC)
                        nc.scalar.dma_start(wq_sb[:, wsl, :], wq[:, wsl, :])
                        if kc % 2 == 0:
                            c4 = kc // 2
                            ksl = slice(c4 * (KO // 4), (c4 + 1) * (KO // 4))
                            nc.scalar.dma_start(wk_sb[:, ksl, :],
                                                wk[:, ksl, :])
                            nc.scalar.dma_start(wv_sb[:, ksl, :],
                                                wv[:, ksl, :])
                    xt = sb.tile([128, XC, ST], BF16, tag="x", bufs=3)
                    nc.sync.dma_start(xt[:], x[si][:, kc * XC:(kc + 1) * XC])
                    for u in range(XC):
                        ko = kc * XC + u
                        st = (ko == 0)
                        sp = (ko == KO - 1)
                        for g in range(G):
                            nc.tensor.matmul(
                                ps_q[g],
                                wq_sb[:, ko, g * 128:(g + 1) * 128],
                                xt[:, u, :], start=st, stop=sp)
                        nc.tensor.matmul(ps_kv[:, 0, :], wk_sb[:, ko, :],
                                         xt[:, u, :], start=st, stop=sp)
                        nc.tensor.matmul(ps_kv[:, 1, :], wv_sb[:, ko, :],
                                         xt[:, u, :], start=st, stop=sp)
                        drain()
                if si == 0:
                    emit_tables_load()
                # boundary: evict psums on vector, defer the PE-side rope /
                # transpose work into the next phase's stream
                k_raw = sb.tile([128, ST], F32R, tag="k_raw", bufs=2)
                nc.vector.tensor_scalar(k_raw[:], ps_kv[:, 0, :],
                                        bk_sb[:, 0:1], None, Alu.add)
                pending.append(rope_cb(k_raw, k_rot[:, sl], sl))
                for g in range(G):
                    q_raw = sb.tile([128, ST], F32R, tag="q_raw", bufs=6,
                                    name=f"q_raw{g}")
                    nc.vector.tensor_scalar(q_raw[:], ps_q[g],
                                            bq_sb[:, g:g + 1], INV_SQRT_D,
                                            Alu.add, Alu.mult)
                    pending.append(rope_cb(q_raw, q_rot[:, g, sl], sl))
                v_sb = sb.tile([128, ST], BF16, tag="v_sb", bufs=2)
                nc.vector.tensor_scalar(v_sb[:], ps_kv[:, 1, :],
                                        bv_sb[:, 0:1], None, Alu.add)
                pending.append(vt_cb(v_sb, si))

            def emit_attn(si):
                # one softmax pipeline across all G heads of this s-tile:
                # scores pairs stay 3 deep so the exp/select chain hides
                # under later matmuls, even across head boundaries
                nli = (si + 1) * (ST // 128)
                sl = slice(si * ST, (si + 1) * ST)
                hist = []
                acc = {}   # g -> avden pair

                def avden(rec):
                    g, p2, offs = rec
                    if g not in acc:
                        acc[g] = pair(f"ps_avden{g}")
                    ps_av, ps_den = acc[g][:, 0, :], acc[g][:, 1, :]
                    for h in range(2):
                        li, off = offs[h]
                        nc.tensor.matmul(ps_av[:, off:], v_t[:, li, :],
                                         p2[:, h, off:],
                                         start=(li == 0),
                                         stop=(li == nli - 1))
                        nc.tensor.matmul(ps_den[:, off:], ones_bf[:],
                                         p2[:, h, off:],
                                         start=(li == 0),
                                         stop=(li == nli - 1))
                    if offs[1][0] == nli - 1:
                        # head done: one copy frees the psum pair, then the
                        # slow recip/mult chain runs off SBUF
                        ad_sb = sb.tile([128, 2, ST], F32, tag="ad", bufs=2)
                        nc.vector.tensor_copy(ad_sb[:], acc[g][:])
                        recip = sb.tile([128, ST], F32, tag="recip", bufs=2)
                        nc.vector.reciprocal_approx_fast(recip[:],
                                                         ad_sb[:, 1, :])
                        nc.vector.tensor_tensor(out_t[:, g, sl],
                                                ad_sb[:, 0, :],
                                                recip[:], Alu.mult)
                        del acc[g]

                for g in range(G):
                    for pi in range(nli // 2):
                        ps_s2 = pair("ps_s2")
                        offs = []
                        for h in range(2):
                            li = 2 * pi + h
                            j = li - si * (ST // 128)
                            off = 128 * j if j > 0 else 0
                            offs.append((li, off))
                            nc.tensor.matmul(
                                ps_s2[:, h, off:],
                                k_rot[:, li * 128:(li + 1) * 128],
                                q_rot[:, g, si * ST + off:(si + 1) * ST],
                                start=True, stop=True)
                        p2 = sb.tile([128, 2, ST], BF16, tag="p", bufs=4)
                        nc.scalar.activation(p2[:], ps_s2[:], ActF.Exp)
                        for h in range(2):
                            li, off = offs[h]
                            j = li - si * (ST // 128)
                            if j >= 0:
                                # causal within the diagonal s-tile: zero
                                # where s - l < 128j via a static mask
                                # (columns below `off` are never read)
                                nc.vector.tensor_tensor(
                                    p2[:, h, off:], p2[:, h, off:],
                                    cmask[:, j, off:], Alu.mult)
                        hist.append((g, p2, offs))
                        if len(hist) >= 3:
                            avden(hist.pop(0))
                        drain()
                for rec in hist:
                    avden(rec)

            def emit_oproj(si):
                sl = slice(si * ST, (si + 1) * ST)
                for mp in range(MO // 2):
                    ps_y2 = pair("ps_y2")
                    for h in range(2):
                        mo = 2 * mp + h
                        for g in range(G):
                            nc.tensor.matmul(ps_y2[:, h, :],
                                             wo_sb[:, mo, g, :],
                                             out_t[:, g, sl],
                                             start=(g == 0),
                                             stop=(g == G - 1))
                        y_sb = sb.tile([128, ST], BF16, tag="y_sb", bufs=6)
                        if mo % 2 == 0:
                            nc.scalar.activation(y_sb[:], ps_y2[:, h, :],
                                                 ActF.Copy)
                        else:
                            nc.vector.tensor_copy(y_sb[:], ps_y2[:, h, :])
                        if not with_collective and mo < NCORES // 2:
                            # profiling build: these row-blocks are the
                            # local stand-in for the ReduceScatter output
                            nc.sync.dma_start(yout[mo][:, sl], y_sb[:])
                        else:
                            qeng = (nc.sync, nc.scalar, nc.gpsimd)[mo % 3]
                            qeng.dma_start(cc_in[si, mo], y_sb[:])
                        drain()
                if with_collective:
                    # core c receives row-blocks mo = 4c..4c+3 of this s-tile
                    nc.gpsimd.collective_compute(
                        "ReduceScatter",
                        Alu.add,
                        replica_groups=[list(range(NCORES))],
                        ins=[cc_in[si].opt()],
                        outs=[cc_out[si].opt()],
                    )
                    nc.sync.dma_start(yout[:, :, sl], cc_out[si])

            def emit_wo_load():
                for c4 in range(4):
                    msl = slice(c4 * (MO // 4), (c4 + 1) * (MO // 4))
                    nc.scalar.dma_start(wo_sb[:, msl], wo[:, msl])

            emit_qkv(0)
            emit_qkv(1)
            emit_attn(0)
            emit_wo_load()
            emit_qkv(2)
            emit_attn(1)
            emit_oproj(0)
            emit_qkv(3)
            emit_attn(2)
            emit_oproj(1)
            emit_attn(3)
            emit_oproj(2)
            emit_oproj(3)
            while pending:
                pending.pop(0)()

    nc.compile()
    return nc


def _rot_matrix():
    # q_rot = R @ q with rotate_half along D: R @ v = concat(-v[64:], v[:64])
    R = np.zeros((128, 128), np.float32)
    for i in range(64):
        R[i, 64 + i] = -1.0
        R[64 + i, i] = 1.0
    return R


def _prep_in_maps(inputs):
    x = np.ascontiguousarray(np.asarray(inputs["hidden_states"],
                                        np.float32)[0, :, 0, :])
    wq = np.asarray(inputs["wq"], np.float32)
    wk = np.asarray(inputs["wk"], np.float32)
    wv = np.asarray(inputs["wv"], np.float32)
    wo = np.asarray(inputs["wo"], np.float32)
    bq = np.asarray(inputs["bq"], np.float32)
    bk = np.asarray(inputs["bk"], np.float32)
    bv = np.asarray(inputs["bv"], np.float32)
    cos_t = np.ascontiguousarray(np.asarray(inputs["cos_t"],
                                            np.float32)[0, 0])  # (128, S)
    sin_t = np.ascontiguousarray(np.asarray(inputs["sin_t"], np.float32)[0, 0])
    rotT = np.ascontiguousarray(_rot_matrix().T)

    # x tiled as [si, p, ko, ST], contiguous per (si, ko-chunk)
    x_t = np.ascontiguousarray(
        x.reshape(KO, 128, NST, ST).transpose(2, 1, 0, 3).astype(NPBF))
    in_maps = []
    for c in range(NCORES):
        qs = slice(c * G * 128, (c + 1) * G * 128)
        ks = slice(c * 128, (c + 1) * 128)
        wq_t = np.ascontiguousarray(
            wq[qs].T.reshape(KO, 128, G * 128).transpose(1, 0, 2)
            .astype(NPBF))
        wk_t = np.ascontiguousarray(
            wk[ks].T.reshape(KO, 128, 128).transpose(1, 0, 2).astype(NPBF))
        wv_t = np.ascontiguousarray(
            wv[ks].T.reshape(KO, 128, 128).transpose(1, 0, 2).astype(NPBF))
        # wo column shard -> (d, mo, g, m): woT[g*128+d, mo*128+m]
        wo_t = np.ascontiguousarray(
            wo[:, qs].T.reshape(G, 128, MO, 128).transpose(1, 2, 0, 3)
            .astype(NPBF))
        in_maps.append({
            "x": x_t,
            "wq": wq_t,
            "wk": wk_t,
            "wv": wv_t,
            "wo": wo_t,
            "bq": np.ascontiguousarray(bq[qs].reshape(G, 128).T),
            "bk": np.ascontiguousarray(bk[ks][:, None]),
            "bv": np.ascontiguousarray(bv[ks][:, None]),
            "cos": cos_t,
            "sin": sin_t,
            "rot": rotT,
        })
    return in_maps


_NC = None


def _get_nc():
    global _NC
    if _NC is None:
        _NC = build_nc()
    return _NC


def assemble_output(results):
    """Per-s-tile ReduceScatter: core c holds y row-blocks mo = 4c..4c+3."""
    y = np.empty((HID, S), np.float32)
    for c in range(NCORES):
        yc = np.asarray(results[c]["y"], dtype=np.float32)  # [4, 128, S]
        for j in range(yc.shape[0]):
            mo = (NCORES // 2) * c + j
            y[mo * 128:(mo + 1) * 128] = yc[j]
    return y[None, :, None, :]


def kernel(**inputs):
    nc = _get_nc()
    in_maps = _prep_in_maps(inputs)
    res = run_bass_kernel_spmd(nc, in_maps, core_ids=list(range(NCORES)))
    return assemble_output(res.results)


# revision 20
# speedup vs baseline: 1.0065x; 1.0065x over previous
"""Trainium2 Bass kernel for AttentionPatcher (GQA attention block, S=2048).

Sharding: 8-way tensor parallel over KV head groups. Core c owns KV head c
and query heads 4c..4c+3: it computes its Q/K/V projections, RoPE, causal
attention, and a full partial o_proj (wo column shard); a per-s-tile
ReduceScatter(add) over the 8 cores then leaves core c with rows
[512c, 512c+512) of the final output, which the host concatenates.

v3: all matmul operands bf16 (adds ~7e-3 rel err vs the 2e-2 gate), wq/wo
SBUF-resident, phases interleaved (QKV / attention / o_proj) with RoPE and
V-transpose work injected into the next phase's matmul stream, PSUM managed
as 2-bank pair tiles (scores pairs share one exp), chunked DMA to keep the
DGE descriptor generators off the critical path, and approximate reciprocal
for the softmax normalizer.
"""
import os
import sys

import numpy as np

if os.path.isdir("/opt/trn_rl_repo") and "/opt/trn_rl_repo" not in sys.path:
    sys.path.insert(0, "/opt/trn_rl_repo")

import ml_dtypes

import concourse.bacc as bacc
import concourse.mybir as mybir
import concourse.tile as tile
from concourse.bass_utils import run_bass_kernel_spmd
from concourse.masks import make_identity

F32 = mybir.dt.float32
F32R = mybir.dt.float32r
BF16 = mybir.dt.bfloat16
ActF = mybir.ActivationFunctionType
Alu = mybir.AluOpType
NPBF = ml_dtypes.bfloat16

H, KV, D, S = 32, 8, 128, 2048
HID = H * D
NCORES = 8
G = H // KV          # query heads per core
ST = 512             # s-tile size
NST = S // ST        # 4 s-tiles
KO = HID // 128      # 32 contraction subtiles
MO = HID // 128      # 32 output row tiles
XC = 4               # x-tile DMA chunk (ko per descriptor)
INV_SQRT_D = 1.0 / float(np.sqrt(D))


def build_nc(with_collective=True):
    nc = bacc.Bacc("TRN2", target_bir_lowering=False, debug=False)

    x = nc.dram_tensor("x", [NST, 128, KO, ST], BF16, kind="ExternalInput")
    wq = nc.dram_tensor("wq", [128, KO, G * 128], BF16, kind="ExternalInput")
    wk = nc.dram_tensor("wk", [128, KO, 128], BF16, kind="ExternalInput")
    wv = nc.dram_tensor("wv", [128, KO, 128], BF16, kind="ExternalInput")
    wo = nc.dram_tensor("wo", [128, MO, G, 128], BF16, kind="ExternalInput")
    bq = nc.dram_tensor("bq", [128, G], F32, kind="ExternalInput")
    bk = nc.dram_tensor("bk", [128, 1], F32, kind="ExternalInput")
    bv = nc.dram_tensor("bv", [128, 1], F32, kind="ExternalInput")
    cos = nc.dram_tensor("cos", [128, S], F32, kind="ExternalInput")
    sin = nc.dram_tensor("sin", [128, S], F32, kind="ExternalInput")
    rot = nc.dram_tensor("rot", [128, 128], F32R, kind="ExternalInput")
    yout = nc.dram_tensor("y", [NCORES // 2, 128, S], BF16,
                          kind="ExternalOutput")

    with tile.TileContext(nc) as tc:
        with (
            tc.tile_pool(name="const", bufs=1) as const,
            tc.tile_pool(name="sb", bufs=3) as sb,
            tc.tile_pool(name="ps", bufs=1, space="PSUM") as ps,
            tc.tile_pool(name="dram", bufs=1, space="DRAM") as dram,
        ):
            # ---- resident weights/tables ----
            wq_sb = const.tile([128, KO, G * 128], BF16)   # 32K/part
            wk_sb = const.tile([128, KO, 128], BF16)       # 8K
            wv_sb = const.tile([128, KO, 128], BF16)       # 8K
            wo_sb = const.tile([128, MO, G, 128], BF16)    # 32K
            # wk/wv/wq chunks are loaded just-in-time inside emit_qkv(0);
            # cos/sin/rot/bias loads are emitted AFTER emit_qkv(0): QKV(0)
            # already needs wq(8M)+x(4M)+wk/wv(2M) and sits right at the HBM
            # bandwidth budget; these are first used at the qkv(0) boundary
            cos_sb = const.tile([128, S], F32)
            sin_sb = const.tile([128, S], F32)
            rot_sb = const.tile([128, 128], F32R)
            bq_sb = const.tile([128, G], F32)
            bk_sb = const.tile([128, 1], F32)
            bv_sb = const.tile([128, 1], F32)

            def emit_tables_load():
                # small tensors first: the qkv(0) boundary evictions read the
                # biases, and the first rope matmul reads rot
                nc.scalar.dma_start(rot_sb[:], rot[:, :])
                nc.scalar.dma_start(bq_sb[:], bq[:, :])
                nc.scalar.dma_start(bk_sb[:], bk[:, :])
                nc.scalar.dma_start(bv_sb[:], bv[:, :])
                nc.scalar.dma_start(cos_sb[:], cos[:, :])
                nc.scalar.dma_start(sin_sb[:], sin[:, :])

            ones_bf = const.tile([128, 128], BF16)
            nc.vector.memset(ones_bf[:], 1.0)
            ident_f = const.tile([128, 128], F32)
            make_identity(nc, ident_f)
            ident_bf = const.tile([128, 128], BF16)
            nc.vector.tensor_copy(ident_bf[:], ident_f[:])
            # static causal masks for the 4 diagonal l-blocks of an s-tile:
            # mask_j[l, s] = 1 where s - l >= 128j else 0
            cmask = const.tile([128, 4, ST], BF16)
            nc.vector.memset(cmask[:], 1.0)
            for j in range(4):
                nc.gpsimd.affine_select(
                    out=cmask[:, j, :], in_=cmask[:, j, :],
                    compare_op=Alu.is_ge, fill=0.0,
                    base=-128 * j, channel_multiplier=-1,
                    pattern=[[1, ST]],
                )

            # ---- resident activations (all bf16) ----
            k_rot = const.tile([128, S], BF16)             # K, (d, l)
            q_rot = const.tile([128, G, S], BF16)          # Q, (d, g, s)
            v_t = const.tile([128, S // 128, 128], BF16)   # V^T
            out_t = const.tile([128, G, S], BF16)          # attn out

            cc_in = dram.tile([NST, MO, 128, ST], BF16)
            cc_out = dram.tile([NST, NCORES // 2, 128, ST], BF16)

            # PSUM discipline: every psum tile is a [128, 2, ST] f32 "pair"
            # (2 banks); the 4 pair bufs cover all 8 banks. Halves act as
            # independent accumulators.
            def pair(name):
                return ps.tile([128, 2, ST], F32, tag="pair", bufs=4,
                               name=name)

            # deferred PE work (rope matmuls / V transposes) injected into
            # the next phase's matmul stream so the PE queue never stalls on
            # the vector-engine eviction chain at a phase boundary
            pending = []

            def drain(n=1):
                for _ in range(n):
                    if pending:
                        pending.pop(0)()

            def rope_cb(raw, dst_ap, sl):
                def cb():
                    ps_r = pair("ps_rot")
                    nc.tensor.matmul(ps_r[:, 0, :], rot_sb[:], raw[:],
                                     start=True, stop=True)
                    t1 = sb.tile([128, ST], F32, tag="rt1", bufs=2)
                    t2 = sb.tile([128, ST], F32, tag="rt2", bufs=2)
                    # split across engines: gpsimd takes the SBUF-only ops
                    nc.gpsimd.tensor_tensor(t1[:], raw[:], cos_sb[:, sl],
                                            Alu.mult)
                    nc.vector.tensor_tensor(t2[:], ps_r[:, 0, :],
                                            sin_sb[:, sl], Alu.mult)
                    nc.gpsimd.tensor_tensor(dst_ap, t1[:], t2[:], Alu.add)
                return cb

            def vt_cb(v_sb, si):
                def cb():
                    ps_t = ps.tile([128, 4, 128], BF16, tag="pair", bufs=4,
                                   name="ps_t")
                    for j in range(4):
                        nc.tensor.transpose(ps_t[:, j, :],
                                            v_sb[:, j * 128:(j + 1) * 128],
                                            ident_bf[:])
                    nc.vector.tensor_copy(v_t[:, si * 4:(si + 1) * 4, :],
                                          ps_t[:])
                return cb

            def emit_qkv(si):
                sl = slice(si * ST, (si + 1) * ST)
                ps_q01 = pair("ps_q01")
                ps_q23 = pair("ps_q23")
                ps_kv = pair("ps_kv")
                ps_q = [ps_q01[:, 0, :], ps_q01[:, 1, :],
                        ps_q23[:, 0, :], ps_q23[:, 1, :]]
                for kc in range(KO // XC):
                    if si == 0:
                        wsl = slice(kc * XC, (kc + 1) * XC)
                        nc.scalar.dma_start(wq_sb[:, wsl, :], wq[:, wsl, :])
                        if kc % 2 == 0:
                            c4 = kc // 2
                            ksl = slice(c4 * (KO // 4), (c4 + 1) * (KO // 4))
                            nc.scalar.dma_start(wk_sb[:, ksl, :],
                                                wk[:, ksl, :])
                            nc.scalar.dma_start(wv_sb[:, ksl, :],
                                                wv[:, ksl, :])
                    xt = sb.tile([128, XC, ST], BF16, tag="x", bufs=3)
                    nc.sync.dma_start(xt[:], x[si][:, kc * XC:(kc + 1) * XC])
                    for u in range(XC):
                        ko = kc * XC + u
                        st = (ko == 0)
                        sp = (ko == KO - 1)
                        for g in range(G):
                            nc.tensor.matmul(
                                ps_q[g],
                                wq_sb[:, ko, g * 128:(g + 1) * 128],
                                xt[:, u, :], start=st, stop=sp)
                        nc.tensor.matmul(ps_kv[:, 0, :], wk_sb[:, ko, :],
                                         xt[:, u, :], start=st, stop=sp)
                        nc.tensor.matmul(ps_kv[:, 1, :], wv_sb[:, ko, :],
                                         xt[:, u, :], start=st, stop=sp)
                        drain()
                if si == 0:
                    emit_tables_load()
                # boundary: evict psums on vector, defer the PE-side rope /
                # transpose work into the next phase's stream
                k_raw = sb.tile([128, ST], F32R, tag="k_raw", bufs=2)
                nc.vector.tensor_scalar(k_raw[:], ps_kv[:, 0, :],
                                        bk_sb[:, 0:1], None, Alu.add)
                pending.append(rope_cb(k_raw, k_rot[:, sl], sl))
                for g in range(G):
                    q_raw = sb.tile([128, ST], F32R, tag="q_raw", bufs=6,
                                    name=f"q_raw{g}")
                    nc.vector.tensor_scalar(q_raw[:], ps_q[g],
                                            bq_sb[:, g:g + 1], INV_SQRT_D,
                                            Alu.add, Alu.mult)
                    pending.append(rope_cb(q_raw, q_rot[:, g, sl], sl))
                v_sb = sb.tile([128, ST], BF16, tag="v_sb", bufs=2)
                nc.vector.tensor_scalar(v_sb[:], ps_kv[:, 1, :],
                                        bv_sb[:, 0:1], None, Alu.add)
                pending.append(vt_cb(v_sb, si))

            def emit_attn(si):
                # one softmax pipeline across all G heads of this s-tile:
                # scores pairs stay 3 deep so the exp/select chain hides
                # under later matmuls, even across head boundaries
                nli = (si + 1) * (ST // 128)
                sl = slice(si * ST, (si + 1) * ST)
                hist = []
                acc = {}   # g -> avden pair

                def avden(rec):
                    g, p2, offs = rec
                    if g not in acc:
                        acc[g] = pair(f"ps_avden{g}")
                    ps_av, ps_den = acc[g][:, 0, :], acc[g][:, 1, :]
                    for h in range(2):
                        li, off = offs[h]
                        nc.tensor.matmul(ps_av[:, off:], v_t[:, li, :],
                                         p2[:, h, off:],
                                         start=(li == 0),
                                         stop=(li == nli - 1))
                        nc.tensor.matmul(ps_den[:, off:], ones_bf[:],
                                         p2[:, h, off:],
                                         start=(li == 0),
                                         stop=(li == nli - 1))
                    if offs[1][0] == nli - 1:
                        # head done: normalize and release its accumulators
                        recip = sb.tile([128, ST], F32, tag="recip", bufs=2)
                        nc.vector.reciprocal_approx_fast(recip[:], ps_den[:])
                        nc.vector.tensor_tensor(out_t[:, g, sl], ps_av[:],
                                                recip[:], Alu.mult)
                        del acc[g]

                for g in range(G):
                    for pi in range(nli // 2):
                        ps_s2 = pair("ps_s2")
                        offs = []
                        for h in range(2):
                            li = 2 * pi + h
                            j = li - si * (ST // 128)
                            off = 128 * j if j > 0 else 0
                            offs.append((li, off))
                            nc.tensor.matmul(
                                ps_s2[:, h, off:],
                                k_rot[:, li * 128:(li + 1) * 128],
                                q_rot[:, g, si * ST + off:(si + 1) * ST],
                                start=True, stop=True)
                        p2 = sb.tile([128, 2, ST], BF16, tag="p", bufs=4)
                        nc.scalar.activation(p2[:], ps_s2[:], ActF.Exp)
                        for h in range(2):
                            li, off = offs[h]
                            j = li - si * (ST // 128)
                            if j >= 0:
                                # causal within the diagonal s-tile: zero
                                # where s - l < 128j via a static mask
                                # (columns below `off` are never read)
                                nc.vector.tensor_tensor(
                                    p2[:, h, off:], p2[:, h, off:],
                                    cmask[:, j, off:], Alu.mult)
                        hist.append((g, p2, offs))
                        if len(hist) >= 3:
                            avden(hist.pop(0))
                        drain()
                for rec in hist:
                    avden(rec)

            def emit_oproj(si):
                sl = slice(si * ST, (si + 1) * ST)
                for mp in range(MO // 2):
                    ps_y2 = pair("ps_y2")
                    for h in range(2):
                        mo = 2 * mp + h
                        for g in range(G):
                            nc.tensor.matmul(ps_y2[:, h, :],
                                             wo_sb[:, mo, g, :],
                                             out_t[:, g, sl],
                                             start=(g == 0),
                                             stop=(g == G - 1))
                        y_sb = sb.tile([128, ST], BF16, tag="y_sb", bufs=6)
                        if mo % 2 == 0:
                            nc.scalar.activation(y_sb[:], ps_y2[:, h, :],
                                                 ActF.Copy)
                        else:
                            nc.vector.tensor_copy(y_sb[:], ps_y2[:, h, :])
                        if not with_collective and mo < NCORES // 2:
                            # profiling build: these row-blocks are the
                            # local stand-in for the ReduceScatter output
                            nc.sync.dma_start(yout[mo][:, sl], y_sb[:])
                        else:
                            qeng = (nc.sync, nc.scalar, nc.gpsimd)[mo % 3]
                            qeng.dma_start(cc_in[si, mo], y_sb[:])
                        drain()
                if with_collective:
                    # core c receives row-blocks mo = 4c..4c+3 of this s-tile
                    nc.gpsimd.collective_compute(
                        "ReduceScatter",
                        Alu.add,
                        replica_groups=[list(range(NCORES))],
                        ins=[cc_in[si].opt()],
                        outs=[cc_out[si].opt()],
                    )
                    nc.sync.dma_start(yout[:, :, sl], cc_out[si])

            def emit_wo_load():
                for c4 in range(4):
                    msl = slice(c4 * (MO // 4), (c4 + 1) * (MO // 4))
                    nc.scalar.dma_start(wo_sb[:, msl], wo[:, msl])

            emit_qkv(0)
            emit_qkv(1)
            emit_attn(0)
            emit_wo_load()
            emit_qkv(2)
            emit_attn(1)
            emit_oproj(0)
            emit_qkv(3)
            emit_attn(2)
            emit_oproj(1)
            emit_attn(3)
            emit_oproj(2)
            emit_oproj(3)
            while pending:
                pending.pop(0)()

    nc.compile()
    return nc


def _rot_matrix():
    # q_rot = R @ q with rotate_half along D: R @ v = concat(-v[64:], v[:64])
    R = np.zeros((128, 128), np.float32)
    for i in range(64):
        R[i, 64 + i] = -1.0
        R[64 + i, i] = 1.0
    return R


def _prep_in_maps(inputs):
    x = np.ascontiguousarray(np.asarray(inputs["hidden_states"],
                                        np.float32)[0, :, 0, :])
    wq = np.asarray(inputs["wq"], np.float32)
    wk = np.asarray(inputs["wk"], np.float32)
    wv = np.asarray(inputs["wv"], np.float32)
    wo = np.asarray(inputs["wo"], np.float32)
    bq = np.asarray(inputs["bq"], np.float32)
    bk = np.asarray(inputs["bk"], np.float32)
    bv = np.asarray(inputs["bv"], np.float32)
    cos_t = np.ascontiguousarray(np.asarray(inputs["cos_t"],
                                            np.float32)[0, 0])  # (128, S)
    sin_t = np.ascontiguousarray(np.asarray(inputs["sin_t"], np.float32)[0, 0])
    rotT = np.ascontiguousarray(_rot_matrix().T)

    # x tiled as [si, p, ko, ST], contiguous per (si, ko-chunk)
    x_t = np.ascontiguousarray(
        x.reshape(KO, 128, NST, ST).transpose(2, 1, 0, 3).astype(NPBF))
    in_maps = []
    for c in range(NCORES):
        qs = slice(c * G * 128, (c + 1) * G * 128)
        ks = slice(c * 128, (c + 1) * 128)
        wq_t = np.ascontiguousarray(
            wq[qs].T.reshape(KO, 128, G * 128).transpose(1, 0, 2)
            .astype(NPBF))
        wk_t = np.ascontiguousarray(
            wk[ks].T.reshape(KO, 128, 128).transpose(1, 0, 2).astype(NPBF))
        wv_t = np.ascontiguousarray(
            wv[ks].T.reshape(KO, 128, 128).transpose(1, 0, 2).astype(NPBF))
        # wo column shard -> (d, mo, g, m): woT[g*128+d, mo*128+m]
        wo_t = np.ascontiguousarray(
            wo[:, qs].T.reshape(G, 128, MO, 128).transpose(1, 2, 0, 3)
            .astype(NPBF))
        in_maps.append({
            "x": x_t,
            "wq": wq_t,
            "wk": wk_t,
            "wv": wv_t,
            "wo": wo_t,
            "bq": np.ascontiguousarray(bq[qs].reshape(G, 128).T),
            "bk": np.ascontiguousarray(bk[ks][:, None]),
            "bv": np.ascontiguousarray(bv[ks][:, None]),
            "cos": cos_t,
            "sin": sin_t,
            "rot": rotT,
        })
    return in_maps


_NC = None


def _get_nc():
    global _NC
    if _NC is None:
        _NC = build_nc()
    return _NC


def assemble_output(results):
    """Per-s-tile ReduceScatter: core c holds y row-blocks mo = 4c..4c+3."""
    y = np.empty((HID, S), np.float32)
    for c in range(NCORES):
        yc = np.asarray(results[c]["y"], dtype=np.float32)  # [4, 128, S]
        for j in range(yc.shape[0]):
            mo = (NCORES // 2) * c + j
            y[mo * 128:(mo + 1) * 128] = yc[j]
    return y[None, :, None, :]


def kernel(**inputs):
    nc = _get_nc()
    in_maps = _prep_in_maps(inputs)
    res = run_bass_kernel_spmd(nc, in_maps, core_ids=list(range(NCORES)))
    return assemble_output(res.results)
